# revision 50
# baseline (speedup 1.0000x reference)
"""Trainium2 Bass kernel for nn_NeuroKernel_69956427318000.

Computes, for x [768] and an MLP (2->1024 sigmoid ->128 relu ->1):
    v(i,j) = MLP(x[i], x[j]) for all upper-triangular pairs j >= i
    K = upper-triangular matrix of v (rest zeros)
    return K.T @ K

Strategy (8-core SPMD, single NEFF launch):
  - ROW sharding of K: core c owns rows i = 8*r + c. Row windows are padded
    to a uniform per-window length (the <=7 pad entries land strictly below
    the diagonal inside the diagonal 128-block and are masked out).
  - Window order: rows 0..47 longest-first, then rows 95..49 shortest-first,
    then row 48 last (its 384-value window is exactly the epilogue chunk, so
    the tail has a single scatter DMA). Rows 0..47 (= K tiles 0-2, needed by
    every K^T K contraction) finish by super-block 27 and are AllGathered
    mid-loop; 9 of the 12 C output blocks are computed during the main loop.
  - Host gathers x[i]/x[j] per pair into a [37, 2, 1024] feed per core
    (73 chunks of 512 pairs; 36 full super-blocks + one epilogue chunk).
  - Device: 3-layer MLP fused on-chip, fp32r matmuls (full PE rate), sigmoid
    on the scalar engine (the bottleneck; ACT streams gap-free), v written
    bf16 into a flat SBUF ring and scattered per-row into a pre-zeroed DRAM
    fragment (bf16 K halves the exchange payload; K^T K runs bf16 x bf16
    with fp32 PSUM accumulation, ~5e-4 overall rel err). Row sharding means
    gathered fragments ARE K row-tiles: one strided DMA per 128-row tile
    un-permutes them (no PE transposes).
  - Tail: the last 3 C blocks pre-accumulate their K-tile-0..2 contributions
    while the final AllGather flies, then finish with 6 matmuls.
"""

import sys

sys.path.insert(0, "/opt/trn_rl_repo")

import numpy as np

try:  # persistent NEFF/executable cache across processes
    import jax

    jax.config.update("jax_compilation_cache_dir", "/tmp/jax_neff_cache")
    jax.config.update("jax_persistent_cache_min_compile_time_secs", 0.0)
    jax.config.update("jax_persistent_cache_min_entry_size_bytes", 0)
except Exception:
    pass

import concourse.bass as bass
import concourse.mybir as mybir
import concourse.tile as tile
from concourse import bacc, bass_utils

N = 768
NCORES = 8
CHUNK = 512
NCHUNKS = 73  # 36 full super-blocks of 2 chunks + 1 epilogue chunk
NSB = 37  # loop iterations; s = 36 is the half-size epilogue
NTILES = N // 128  # 6
RING = 9216  # flat v ring in SBUF (18 chunks)

F32 = mybir.dt.float32
F32R = mybir.dt.float32r
BF16 = mybir.dt.bfloat16

# Window p handles K row r = R_ORDER[p] of this core's shard (global row
# i = 8r + c), covering cols [8r, 768), padded length L_p = 768 - 8r.
R_ORDER = list(range(48)) + [143 - p for p in range(48, 95)] + [48]
_L = [N - 8 * R_ORDER[p] for p in range(96)]
_F = np.concatenate([[0], np.cumsum(_L)]).astype(int)  # _F[96] = 37248
P_CORE = int(_F[96])  # padded to NCHUNKS*CHUNK = 37376

# windows whose flat range completes within chunk k
_ROWS_DONE = [[] for _ in range(NCHUNKS)]
for _p in range(96):
    _ROWS_DONE[(int(_F[_p + 1]) - 1) // CHUNK].append(_p)

AG_SB = (int(_F[48]) - 1) // (2 * CHUNK)  # = 27: rows 0..47 done in SB 27

# C = K^T K blocks: output block (mi, nb) covers C[128*mi:+128, 384*nb:+384]
# and contracts K row-tiles ki <= min(mi, 3*nb+2) (the rest are all-zero).
# Blocks needing only tiles 0-2 run mid-loop right after the first AllGather.
MID_BLOCKS = [(mi, 0) for mi in range(6)] + [(mi, 1) for mi in range(3)]
TAIL_BLOCKS = [(3, 1), (4, 1), (5, 1)]


def build_module(with_collective=True):
    nc = bacc.Bacc(
        "TRN2", target_bir_lowering=False, debug=False, num_devices=NCORES
    )
    pairs_d = nc.dram_tensor(
        "pairs", [NSB, 2, 2 * CHUNK], F32R, kind="ExternalInput"
    ).ap()
    w1t_d = nc.dram_tensor("w1t", [2, 1024], F32R, kind="ExternalInput").ap()
    w2t_d = nc.dram_tensor("w2t", [1024, 128], F32R, kind="ExternalInput").ap()
    w3t_d = nc.dram_tensor("w3t", [128, 1], F32R, kind="ExternalInput").ap()
    b1r_d = nc.dram_tensor("b1r", [128, 8], F32, kind="ExternalInput").ap()
    b2r_d = nc.dram_tensor("b2r", [128, 1], F32, kind="ExternalInput").ap()
    b3r_d = nc.dram_tensor("b3r", [1, 1], F32, kind="ExternalInput").ap()
    out_d = nc.dram_tensor("out", [N, N], F32, kind="ExternalOutput").ap()

    with tile.TileContext(nc) as tc:
        with (
            tc.tile_pool(name="const", bufs=1) as const,
            tc.tile_pool(name="rhsp", bufs=4) as rhsp,
            tc.tile_pool(name="h1p", bufs=6) as h1p,
            tc.tile_pool(name="h2sp", bufs=3) as h2sp,
            tc.tile_pool(name="csb", bufs=12) as csb,
            tc.tile_pool(name="dram", bufs=1, space="DRAM") as dram,
        ):
            # --- constants / weights ---
            w1s = const.tile([2, 1024], F32R, name="w1s")
            w2s = const.tile([128, 1024], F32R, name="w2s")
            w3s = const.tile([128, 1], F32R, name="w3s")
            b1s = const.tile([128, 8], F32, name="b1s")
            b2s = const.tile([128, 1], F32, name="b2s")
            b3s = const.tile([1, 1], F32, name="b3s")
            flat = const.tile([1, RING], BF16, name="flat")
            zsrc = const.tile([96, N], BF16, name="zsrc")
            # all 6 K row-tiles in one tile: tile it lives at cols [768*it, +768)
            ks_all = const.tile([128, NTILES * N], BF16, name="ks_all")

            def kssl(it, lo, hi):  # K[128*it:+128, lo:hi] (fp32r)
                return ks_all[:, N * it + lo : N * it + hi]

            # First-needed loads go first (w1/b1 + the first two pair
            # super-blocks gate the first sigmoids).
            # rhs feed DMAs ride the Pool-engine SWDGE path, parallel to the
            # HWDGE queue that carries weights/scatters.
            nc.sync.dma_start(w1s[:], w1t_d[:])
            rhs_tiles = {}
            for s in range(3):
                rhs_tiles[s] = rhsp.tile([2, 2 * CHUNK], F32R, name="rhs")
                nc.gpsimd.dma_start(rhs_tiles[s][:], pairs_d[s, :, :])
            nc.sync.dma_start(b1s[:], b1r_d[:])

            # Warmup activation: pulls the sigmoid table load off the
            # critical path (overlaps the initial DMAs).
            warm = const.tile([1, 1], F32, name="warm")
            nc.vector.memset(warm[:], 0.0)
            nc.scalar.activation(
                warm[:], warm[:], mybir.ActivationFunctionType.Sigmoid
            )

            # Remaining weights: w2 as one strided DMA ([1024,128] -> the
            # [128, 8x128] SBUF layout), then the small tensors.
            nc.sync.dma_start(
                w2s[:].rearrange("p (k c) -> p k c", k=8),
                w2t_d[:].rearrange("(k p) c -> p k c", k=8),
            )
            nc.sync.dma_start(w3s[:], w3t_d[:])
            nc.sync.dma_start(b2s[:], b2r_d[:])
            nc.sync.dma_start(b3s[:], b3r_d[:])
            nc.vector.memset(zsrc[:], 0.0)

            # Upper-keep 0/1 mask (keep f >= p): zeroes the <=7 below-diagonal
            # pad entries, which always land in the diagonal 128-blocks.
            mtri = const.tile([128, 128], BF16, name="mtri")
            nc.gpsimd.memset(mtri[:], 1.0)
            nc.gpsimd.affine_select(
                out=mtri[:],
                in_=mtri[:],
                compare_op=mybir.AluOpType.is_ge,
                fill=0.0,
                base=0,
                pattern=[[1, 128]],
                channel_multiplier=-1,
            )

            # Pre-zeroed DRAM row-fragments (scatter fills the real spans).
            ctd_a = dram.tile([48, N], BF16, name="ctd_a")  # K rows 0..47
            ctd_b = dram.tile([48, N], BF16, name="ctd_b")  # K rows 48..95
            nc.sync.dma_start(ctd_a[:], zsrc[0:48, :])
            nc.sync.dma_start(ctd_b[:], zsrc[48:96, :])

            def emit_scatter(p):
                r = R_ORDER[p]
                L = _L[p]
                if r < 48:
                    dst = ctd_a[r : r + 1, N - L : N]
                else:
                    dst = ctd_b[r - 48 : r - 47, N - L : N]
                s0 = int(_F[p]) % RING
                if s0 + L <= RING:
                    nc.sync.dma_start(dst, flat[0:1, s0 : s0 + L])
                else:
                    cut = RING - s0
                    nc.sync.dma_start(dst[:, 0:cut], flat[0:1, s0:RING])
                    nc.sync.dma_start(dst[:, cut:L], flat[0:1, 0 : L - cut])

            # AllGather one [48, N] fragment and un-permute into three K
            # row-tiles: kss[it][q, :] = K[128*it + q, :] with q = 8u + v
            # coming from ct_all row 48v + 16(it - tbase) + u.
            def emit_exchange(ctd_h, tbase, tag):
                if with_collective:
                    ct_all = dram.tile(
                        [NCORES * 48, N], BF16, addr_space="Shared",
                        name=f"cta_{tag}",
                    )
                    nc.gpsimd.collective_compute(
                        "AllGather",
                        mybir.AluOpType.bypass,
                        replica_groups=[list(range(NCORES))],
                        ins=[ctd_h.opt()],
                        outs=[ct_all.opt()],
                    )
                else:  # timing-sim stand-in: one broadcast copy
                    ct_all = dram.tile([NCORES * 48, N], BF16, name=f"cta_{tag}")
                    nc.sync.dma_start(
                        ct_all[:].rearrange("(v t) f -> v t f", v=NCORES),
                        ctd_h[:]
                        .rearrange("t (o f) -> o t f", o=1)
                        .broadcast_to([NCORES, 48, N]),
                    )
                # kss tile partition q = 8u + v in natural order, so the dst
                # is a plain [128, 768] slice; the permutation lives entirely
                # in the DRAM-side source AP (rows 48v + 16k + u).
                src = ct_all[:].rearrange("(v t u) f -> t u v f", v=8, t=3)
                for k in range(3):
                    it = tbase + k
                    nc.sync.dma_start(
                        ks_all[:, N * it : N * (it + 1)], src[k]
                    )
                    dslice = ks_all[
                        :, N * it + 128 * it : N * it + 128 * (it + 1)
                    ]
                    nc.vector.tensor_tensor(
                        dslice, dslice, mtri[:], op=mybir.AluOpType.mult
                    )

            def emit_ktk(cpool, mi, nb, ki_lo, ki_hi, start, stop, cps=None):
                if cps is None:
                    cps = cpool.tile([128, 384], F32, name="cps")
                for ki in range(ki_lo, ki_hi + 1):
                    nc.tensor.matmul(
                        cps[:],
                        kssl(ki, 128 * mi, 128 * (mi + 1)),
                        kssl(ki, 384 * nb, 384 * (nb + 1)),
                        start=(start and ki == ki_lo),
                        stop=(stop and ki == ki_hi),
                    )
                return cps

            def emit_cout(mi, nb, cps, engine):
                cs = csb.tile([128, 384], F32, name="cs")
                if engine == "act":
                    nc.scalar.copy(cs[:], cps[:])
                else:
                    nc.vector.tensor_copy(cs[:], cps[:])
                nc.sync.dma_start(
                    out_d[128 * mi : 128 * (mi + 1), 384 * nb : 384 * (nb + 1)],
                    cs[:],
                )

            # --- main MLP loop (36 full SBs of 1024 pairs + 512 epilogue) ---
            with (
                tc.tile_pool(name="prep", bufs=3, space="PSUM") as prep,
                tc.tile_pool(name="hvp", bufs=2, space="PSUM") as hvp,
            ):
                for s in range(NSB):
                    # epilogue chunk has only 384 real pairs (row 48's window)
                    chw = [CHUNK, CHUNK] if s < NSB - 1 else [384]
                    CH = sum(chw)
                    nch = len(chw)
                    rhs = rhs_tiles.pop(s)
                    if s + 3 < NSB:
                        rhs_tiles[s + 3] = rhsp.tile(
                            [2, 2 * CHUNK], F32R, name="rhs"
                        )
                        nc.gpsimd.dma_start(
                            rhs_tiles[s + 3][:], pairs_d[s + 3, :, :]
                        )

                    h2ps = [
                        hvp.tile([128, w], F32, name="h2ps") for w in chw
                    ]
                    for f in range(8):
                        pre = prep.tile([128, CH], F32, name="pre")
                        for t, w in enumerate(chw):
                            nc.tensor.matmul(
                                pre[:, CHUNK * t : CHUNK * t + w],
                                w1s[:, 128 * f : 128 * (f + 1)],
                                rhs[:, CHUNK * t : CHUNK * t + w],
                                start=True,
                                stop=True,
                            )
                        h1 = h1p.tile([128, CH], F32R, name="h1")
                        nc.scalar.activation(
                            h1[:],
                            pre[:],
                            mybir.ActivationFunctionType.Sigmoid,
                            bias=b1s[:, f : f + 1],
                            scale=1.0,
                        )
                        for t, w in enumerate(chw):
                            nc.tensor.matmul(
                                h2ps[t][:],
                                w2s[:, 128 * f : 128 * (f + 1)],
                                h1[:, CHUNK * t : CHUNK * t + w],
                                start=(f == 0),
                                stop=(f == 7),
                            )

                    for t, w in enumerate(chw):
                        k = 2 * s + t
                        h2s = h2sp.tile([128, w], F32R, name="h2s")
                        nc.vector.tensor_scalar(
                            h2s[:],
                            h2ps[t][:],
                            b2s[:],
                            0.0,
                            op0=mybir.AluOpType.add,
                            op1=mybir.AluOpType.max,
                        )
                        # shares the h2ps tag: v reuses the slot of the h2
                        # accumulator that relu just drained (8-bank budget)
                        v = hvp.tile([1, w], F32, name="h2ps")
                        nc.tensor.matmul(
                            v[:], w3s[:], h2s[:], start=True, stop=True
                        )
                        base = (k * CHUNK) % RING
                        nc.vector.tensor_scalar(
                            flat[0:1, base : base + w],
                            v[:],
                            b3s[:],
                            None,
                            op0=mybir.AluOpType.add,
                        )
                        for p in _ROWS_DONE[k]:
                            emit_scatter(p)

                    if s == AG_SB:
                        # K rows 0..47 (tiles 0-2, needed by every C block)
                        # are complete: exchange them during the main loop.
                        emit_exchange(ctd_a, 0, "a")

            # --- tail: everything that needs only tiles 0-2 (9 full C blocks
            # + the tile-0..2 partial sums of the last 3) runs on a warm PE
            # while the final AllGather flies; then 6 matmuls finish. ---
            with tc.tile_pool(name="cpp2", bufs=6, space="PSUM") as cpp2:
                emit_exchange(ctd_b, 3, "b")
                parts = {}
                for mi, nb in TAIL_BLOCKS:
                    parts[mi] = emit_ktk(cpp2, mi, nb, 0, 2, True, False)
                for mi, nb in MID_BLOCKS:
                    kmax = min(mi, 3 * nb + 2)
                    cps = emit_ktk(cpp2, mi, nb, 0, kmax, True, True)
                    emit_cout(mi, nb, cps, "dve")
                # ki-major: each of the last 3 blocks closes (stop + copy +
                # store) as soon as its highest K tile lands from the gather.
                for ki in (3, 4, 5):
                    for mi, nb in TAIL_BLOCKS:
                        if ki > mi:
                            continue
                        emit_ktk(
                            cpp2, mi, nb, ki, ki, False, ki == mi,
                            cps=parts[mi],
                        )
                        if ki == mi:
                            emit_cout(mi, nb, parts[mi], "act" if mi % 2 else "dve")
    nc.compile()
    return nc


_CACHED = None


def _get_module():
    global _CACHED
    if _CACHED is None:
        _CACHED = build_module()
    return _CACHED


def _host_inputs(x, W1, b1, W2, b2, W3, b3):
    x = np.asarray(x, dtype=np.float32)
    w1t = np.ascontiguousarray(np.asarray(W1, np.float32).T)  # [2, 1024]
    w2t = np.ascontiguousarray(np.asarray(W2, np.float32).T)  # [1024, 128]
    w3t = np.ascontiguousarray(np.asarray(W3, np.float32).T)  # [128, 1]
    b1r = np.ascontiguousarray(np.asarray(b1, np.float32).reshape(8, 128).T)
    b2r = np.asarray(b2, np.float32).reshape(128, 1)
    b3r = np.asarray(b3, np.float32).reshape(1, 1)

    # window p covers K row i = 8*R_ORDER[p] + c, cols j in [8*R_ORDER[p], 768)
    jj = np.concatenate(
        [np.arange(8 * R_ORDER[p], N, dtype=np.int64) for p in range(96)]
    )
    ii_base = np.concatenate(
        [np.full(_L[p], 8 * R_ORDER[p], dtype=np.int64) for p in range(96)]
    )
    feed_len = NSB * 2 * CHUNK  # 37888 (tail past P_CORE is junk)
    jj = np.concatenate([jj, np.zeros(feed_len - P_CORE, dtype=np.int64)])
    ii_base = np.concatenate(
        [ii_base, np.zeros(feed_len - P_CORE, dtype=np.int64)]
    )

    in_maps = []
    for c in range(NCORES):
        ii = np.minimum(ii_base + c, N - 1)
        xi = x[ii].reshape(NSB, 2 * CHUNK)
        xj = x[jj].reshape(NSB, 2 * CHUNK)
        pairs = np.ascontiguousarray(
            np.stack([xi, xj], axis=1), dtype=np.float32
        )
        in_maps.append(
            {
                "pairs": pairs,
                "w1t": w1t,
                "w2t": w2t,
                "w3t": w3t,
                "b1r": b1r,
                "b2r": b2r,
                "b3r": b3r,
            }
        )
    return in_maps


def run(x, W1, b1, W2, b2, W3, b3, trace=False, **trace_kwargs):
    nc = _get_module()
    in_maps = _host_inputs(x, W1, b1, W2, b2, W3, b3)
    res = bass_utils.run_bass_kernel_spmd(
        nc, in_maps, core_ids=list(range(NCORES)), trace=trace, **trace_kwargs
    )
    return np.asarray(res.results[0]["out"], dtype=np.float32), res


def kernel(x, W1, b1, W2, b2, W3, b3):
    out, _ = run(x, W1, b1, W2, b2, W3, b3)
    return out
